# revision 42
# baseline (speedup 1.0000x reference)
"""DGCN-GRU message passing (nn_DGCNGRU) on 8 Trainium2 NeuronCores.

Strategy (sizes hardcoded for N=50000, K=8, H=128, DEPTH=5, 8 cores):
  - Messages are sharded 6250 rows/core (data-parallel over the message
    dim); the small weights are replicated, all PE matmuls run in fp16.
  - The evolving hidden state h lives in DRAM as a [50000, 128] fp16
    row-major table. Two tables alternate per depth step so the next
    step's table can be AllGathered piecewise (4 pieces per step) while
    the current step's gathers still read the old one.
  - The neighbor gather h[bgraph] uses gpsimd dma_gather with a pair
    trick: index = bgraph>>1 (fits the int16 index limit), each
    descriptor moves 2 rows (512 B = full DMA descriptor efficiency),
    transpose=True lands the two candidate rows as two [128, items]
    SBUF planes; ONE in-place copy_predicated against a uint8 parity
    mask overwrites the lo plane with hi where needed. Gathers are 896
    indices each, 2 per chunk, rotated over 4 SWDGE queues with 6
    chunk-buffers in flight (queue q runs on Q7 core pair q, so 4
    queues generate descriptors in parallel).
  - Compute stays in transposed space [h on partitions, messages on
    the free dim]: r2 = U_r @ h_nei on PE with the depth-invariant r
    precompute added through an accumulated identity matmul;
    sigmoid/tanh on ACT with the torch biases as per-partition bias
    operands; both k-sums (sum_h, sum of r*h_nei) via single DVE
    pool_avg ops with the *8 folded into W_z/W_h host-side.
  - z/h precomputes (W*fmess) stay resident in SBUF across all steps.
  - Step 1 skips the gather entirely (h starts at zero).

kernel(**inputs) takes the full unsharded numpy inputs and returns the
full [50000, 128] float32 output. The Bass program is built and compiled
once per process and reused across calls (it depends only on shapes).
"""


from dataclasses import dataclass

import numpy as np

import concourse.bass as bass
import concourse.mybir as mybir

F16 = mybir.dt.float16
F32 = mybir.dt.float32
U8 = mybir.dt.uint8
I16 = mybir.dt.int16
AF = mybir.ActivationFunctionType
ALU = mybir.AluOpType


@dataclass
class Cfg:
    n_mess: int = 50000
    n_cores: int = 8
    depth: int = 5
    k: int = 8
    h: int = 128
    cn: int = 224          # n's per chunk; 2 gathers of 896 idxs each
    chunks_per_piece: int = 4   # AllGather piece granularity

    @property
    def n_loc(self):
        assert self.n_mess % self.n_cores == 0
        return self.n_mess // self.n_cores

    @property
    def n_pad(self):
        return ((self.n_loc + 127) // 128) * 128

    @property
    def items(self):
        return self.n_pad * self.k

    @property
    def chunks(self):
        """List of (n_offset, cn) chunk descriptors covering n_pad."""
        out = []
        off = 0
        while off < self.n_pad:
            cn = min(self.cn, self.n_pad - off)
            assert (cn * self.k) % 256 == 0
            out.append((off, cn))
            off += cn
        return out

    @property
    def piece_starts(self):
        """Local row start of each AllGather piece (128-aligned). The
        last piece is small so the step-boundary AllGather tail is tiny."""
        step = self.chunks_per_piece * self.cn
        out = [s for s in range(0, self.n_pad - step, step)]
        # split the final group so only a sliver waits on the last chunk
        out.append(self.n_pad - step)
        out.append(((self.n_pad - self.cn) // 128) * 128)
        return out

    @property
    def piece_rows(self):
        """Real (unpadded) local rows per piece."""
        starts = self.piece_starts
        ends = starts[1:] + [self.n_pad]
        return [min(e, self.n_loc) - s for s, e in zip(starts, ends)]

    @property
    def piece_bases(self):
        """Global table base row of each piece (piece-major layout)."""
        out, acc = [], 0
        for pr in self.piece_rows:
            out.append(acc)
            acc += self.n_cores * pr
        return out


def host_inputs(fmess, bgraph, W_z, b_z, W_r, U_r, b_Ur, W_h, b_h, cfg: Cfg):
    """Build the per-core in_map numpy dict."""
    n, h = cfg.n_mess, cfg.h
    nl, npad, k = cfg.n_loc, cfg.n_pad, cfg.k
    fmess = np.asarray(fmess, np.float32)
    bgraph = np.asarray(bgraph)

    shared = {
        "wrT": np.ascontiguousarray(W_r.T).astype(np.float16),
        "wz1T": np.ascontiguousarray(W_z[:, :h].T).astype(np.float16),
        "wh1T": np.ascontiguousarray(W_h[:, :h].T).astype(np.float16),
        "urT": np.ascontiguousarray(U_r.T).astype(np.float16),
        "wz2T": np.ascontiguousarray(W_z[:, h:].T).astype(np.float16),
        "wh2T": np.ascontiguousarray(W_h[:, h:].T).astype(np.float16),
        "ident16": np.eye(h, dtype=np.float16),
        "bz": np.asarray(b_z, np.float32).reshape(h, 1),
        "bur": np.asarray(b_Ur, np.float32).reshape(h, 1),
        "bh": np.asarray(b_h, np.float32).reshape(h, 1),
    }

    # piece-major table row map: the AllGather runs per piece with a
    # contiguous destination, so table rows are ordered
    # [piece0: core0 rows | core1 rows | ...][piece1: ...].
    prows = cfg.piece_rows          # local row count per piece
    pbase = cfg.piece_bases         # global table base per piece
    m = np.arange(n)
    core = m // nl
    r = m % nl
    pidx_of_r = np.searchsorted(np.asarray(cfg.piece_starts), r,
                                side="right") - 1
    off = r - np.asarray(cfg.piece_starts)[pidx_of_r]
    rowmap = (np.asarray(pbase)[pidx_of_r]
              + core * np.asarray(prows)[pidx_of_r] + off)

    in_maps = []
    for c in range(cfg.n_cores):
        sl = slice(c * nl, (c + 1) * nl)
        fT = np.zeros((h, npad), np.float16)
        fT[:, :nl] = fmess[sl].T.astype(np.float16)
        bg = np.zeros((npad, k), np.int64)
        bg[:nl] = rowmap[np.asarray(bgraph[sl])]
        # item stream: per (chunk, gather-half) k-major [k, 112] so the
        # on-chip k-sum trees are contiguous-halves adds (2x DVE mode)
        gn = 112
        flat = bg.reshape(npad // gn, gn, k).transpose(0, 2, 1).reshape(-1)
        pidx = (flat >> 1).astype(np.int16)
        idx = np.tile(pidx.reshape(cfg.items // 16, 16).T, (8, 1))
        mask = np.broadcast_to(
            (flat & 1).astype(np.uint8), (128, cfg.items)).copy()
        maskcol = np.ones((h, 1), np.float32)
        if c == 0:
            maskcol[:, 0] = 0.0
        in_maps.append({
            "fmessT": fT,
            "idx": idx,
            "mask": mask,
            "maskcol": maskcol,
            **shared,
        })
    return in_maps


def declare_io(nc, cfg: Cfg):
    h, npad = cfg.h, cfg.n_pad
    mk = lambda name, shape, dt: nc.dram_tensor(
        name, list(shape), dt, kind="ExternalInput").ap()
    ins = {
        "fmessT": mk("fmessT", (h, npad), F16),
        "idx": mk("idx", (128, cfg.items // 16), I16),
        "mask": mk("mask", (128, cfg.items), U8),
        "maskcol": mk("maskcol", (h, 1), F32),
        "wrT": mk("wrT", (h, h), F16),
        "wz1T": mk("wz1T", (h, h), F16),
        "wh1T": mk("wh1T", (h, h), F16),
        "urT": mk("urT", (h, h), F16),
        "wz2T": mk("wz2T", (h, h), F16),
        "wh2T": mk("wh2T", (h, h), F16),
        "ident16": mk("ident16", (h, h), F16),
        "bz": mk("bz", (h, 1), F32),
        "bur": mk("bur", (h, 1), F32),
        "bh": mk("bh", (h, 1), F32),
    }
    out = nc.dram_tensor("hT", [h, npad], F16, kind="ExternalOutput").ap()
    return ins, out


def build_gru(tc, out_hT, ins, cfg: Cfg):
    nc = tc.nc
    h, k, npad, nl = cfg.h, cfg.k, cfg.n_pad, cfg.n_loc
    N = cfg.n_mess
    rg = [list(range(cfg.n_cores))]
    chunks = cfg.chunks
    cpp = cfg.chunks_per_piece

    pstarts = cfg.piece_starts
    pends = pstarts[1:] + [npad]
    pbases = cfg.piece_bases
    prows = cfg.piece_rows

    # internal DRAM: double-buffered h table (piece-major row order)
    tables = [
        nc.dram_tensor(f"table{i}", [N, h], F16, kind="Internal",
                       addr_space="Shared").ap()
        for i in range(2)
    ]
    shards = [
        nc.dram_tensor(f"shard{i}", [nl, h], F16, kind="Internal").ap()
        for i in range(2)
    ]
    tab_pairs = [t.rearrange("(p two) hh -> p (two hh)", two=2) for t in tables]

    with (
        tc.tile_pool(name="stat", bufs=1) as stat,
        tc.tile_pool(name="work", bufs=1) as work,
        tc.tile_pool(name="small", bufs=2) as small,
        tc.tile_pool(name="ps_big", bufs=1, space="PSUM") as ps_big,
        tc.tile_pool(name="ps_sm", bufs=1, space="PSUM") as ps_sm,
        tc.tile_pool(name="ps_tr", bufs=2, space="PSUM") as ps_tr,
    ):
        # ---- resident statics ----
        fm_sb = stat.tile([h, npad], F16)
        nc.sync.dma_start(fm_sb[:], ins["fmessT"][:])
        idx_sb = stat.tile([128, cfg.items // 16], I16)
        nc.sync.dma_start(idx_sb[:], ins["idx"][:])
        mask_sb = stat.tile([128, cfg.items], U8)
        nc.sync.dma_start(mask_sb[:], ins["mask"][:])
        rpre_sb = stat.tile([h, npad], F16)
        zpre_sb = stat.tile([h, npad], F16)
        hpre_sb = stat.tile([h, npad], F16)
        h16_full = stat.tile([h, npad], F16)
        w = {}
        for name in ("wrT", "wz1T", "wh1T", "wz2T", "wh2T", "urT",
                     "ident16"):
            w[name] = stat.tile([h, h], F16, tag=name, name=name)
            nc.sync.dma_start(w[name][:], ins[name][:])
        for name in ("bz", "bur", "bh", "maskcol"):
            w[name] = stat.tile([h, 1], F32, tag=name, name=name)
            nc.sync.dma_start(w[name][:], ins[name][:])
        urT, i16t = w["urT"], w["ident16"]

        def stt(out, in0, in1, op1, scalar=0.0, op0=ALU.bypass):
            nc.vector.scalar_tensor_tensor(out, in0, scalar, in1, op0, op1)

        def piece_collective(step, pi):
            """AllGather a shard piece into this step's write table."""
            tbl = step % 2
            r0, r1 = pstarts[pi], min(pends[pi], nl)
            gb = pbases[pi]
            nc.gpsimd.collective_compute(
                "AllGather", ALU.bypass, replica_groups=rg,
                ins=[shards[tbl][r0:r1, :]],
                outs=[tables[tbl][gb:gb + cfg.n_cores * prows[pi], :]])

        class StepFlush:
            """Streams h16_full into the shard (PE transpose + DMA per
            128-block as columns complete) and dispatches each piece's
            AllGather two chunks after its rows are all DMA'd (so the
            in-order Pool queue hits an already-satisfied wait)."""

            def __init__(self, step):
                self.step = step
                self.next_b = 0      # next 128-col transpose block
                self.next_p = 0      # next piece to queue
                self.pending = []    # (dispatch_at_chunk, piece)

            def after_chunk(self, ci, done_cols):
                tbl = self.step % 2
                while self.pending and ci >= self.pending[0][0]:
                    piece_collective(self.step, self.pending.pop(0)[1])
                while (self.next_b + 1) * 128 <= done_cols:
                    b = self.next_b * 128
                    pst = ps_tr.tile([128, 128], F16)
                    nc.tensor.transpose(pst[:], h16_full[:, b:b + 128],
                                        i16t[:])
                    row = small.tile([128, 128], F16, tag="row")
                    nc.scalar.activation(row[:], pst[:], AF.Copy)
                    rows = max(0, min(nl - b, 128))
                    if rows:
                        nc.sync.dma_start(shards[tbl][b:b + rows, :],
                                          row[:rows, :])
                    self.next_b += 1
                while (self.next_p < len(pstarts)
                       and min(pends[self.next_p], nl) <= self.next_b * 128):
                    self.pending.append((ci + 1, self.next_p))
                    self.next_p += 1

            def finish(self):
                for _, pi in self.pending:
                    piece_collective(self.step, pi)
                self.pending = []
                while self.next_p < len(pstarts):
                    piece_collective(self.step, self.next_p)
                    self.next_p += 1

        # ---- phase 0: precomputes + step 1 (h == 0 before the first step) ----
        flush0 = StepFlush(0)
        for ci, (n0, cn) in enumerate(chunks):
            csl = slice(n0, n0 + cn)
            fr = fm_sb[:, csl]

            ps = ps_sm.tile([h, cn], F32, tag="psg")
            nc.tensor.matmul(ps[:, :cn], w["wrT"][:], fr,
                             start=True, stop=True)
            nc.scalar.activation(rpre_sb[:, csl], ps[:, :cn], AF.Copy)

            psz = ps_sm.tile([h, cn], F32, tag="psz")
            nc.tensor.matmul(psz[:, :cn], w["wz1T"][:], fr,
                             start=True, stop=True)
            nc.scalar.activation(zpre_sb[:, csl], psz[:, :cn], AF.Copy)
            z1 = small.tile([h, cn], F16, tag="z")
            nc.scalar.activation(z1[:, :cn], psz[:, :cn], AF.Sigmoid,
                                 bias=w["bz"][:])

            psh = ps_sm.tile([h, cn], F32, tag="psh")
            nc.tensor.matmul(psh[:, :cn], w["wh1T"][:], fr,
                             start=True, stop=True)
            nc.scalar.activation(hpre_sb[:, csl], psh[:, :cn], AF.Copy)
            ph1 = small.tile([h, cn], F16, tag="ph")
            nc.scalar.activation(ph1[:, :cn], psh[:, :cn], AF.Tanh,
                                 bias=w["bh"][:])

            stt(h16_full[:, csl], z1[:, :cn], ph1[:, :cn], ALU.mult)
            if n0 == 0:
                stt(h16_full[:, 0:1], h16_full[:, 0:1], w["maskcol"][:],
                    ALU.mult)
            flush0.after_chunk(ci, n0 + cn)
        flush0.finish()

        # ---- depth steps 2..depth ----
        gq = [0]
        for step in range(1, cfg.depth):
            last = step == cfg.depth - 1
            src = tab_pairs[(step - 1) % 2]
            flush = StepFlush(step)
            for ci, (n0, cn) in enumerate(chunks):
                csl = slice(n0, n0 + cn)
                citems = cn * k
                ioff = n0 * k

                cg = 896
                gn = 112
                ng = cn // gn
                pairs = []
                for g in range(ng):
                    pg = work.tile([128, 2, cg], F16, tag=f"pair{g}", bufs=4)
                    pairs.append(pg)
                    nc.gpsimd.dma_gather(
                        out_ap=pg[:, :, :],
                        in_ap=src,
                        idxs_ap=idx_sb[:, (ioff + g * cg) // 16:
                                       (ioff + (g + 1) * cg) // 16],
                        num_idxs=cg,
                        num_idxs_reg=cg,
                        elem_size=2 * h,
                        transpose=True,
                        queue_num=gq[0] % 4,
                    )
                    gq[0] += 1

                sumh = small.tile([h, cn], F32, tag="sumh")
                gated = work.tile([128, ng, cg], F16, tag="gated", bufs=2)
                for g in range(ng):
                    pg = pairs[g]
                    gsl = slice(g * gn, (g + 1) * gn)
                    # parity select, in place: lo := where(mask, hi, lo)
                    nc.vector.copy_predicated(
                        pg[:, 0, :],
                        mask_sb[:, ioff + g * cg:ioff + (g + 1) * cg],
                        pg[:, 1, :])
                    hng = pg[:, 0, :]   # [128, cg], k-major: [k=8, n=112]

                    # r2 = rpre (ACT-broadcast into PSUM) + U_r @ hn, sigmoid
                    r16 = work.tile([128, cg], F16, tag=f"r16{g}", bufs=2)
                    rb = rpre_sb[:, n0 + g * gn:n0 + (g + 1) * gn]
                    rb = rb.rearrange("p (one a) -> p one a", one=1)
                    rb = rb.broadcast_to((128, 4, gn))
                    for s0 in (0, cg // 2):
                        psr = ps_big.tile([128, cg // 2], F32, tag="psr",
                                          bufs=2)
                        nc.tensor.matmul(
                            psr[:, :], urT[:], hng[:, s0:s0 + cg // 2],
                            start=True, stop=False)
                        nc.tensor.matmul(psr[:, :], i16t[:], rb,
                                         start=False, stop=True)
                        nc.scalar.activation(r16[:, s0:s0 + cg // 2],
                                             psr[:, :],
                                             AF.Sigmoid, bias=w["bur"][:])

                    stt(gated[:, g, :], r16[:, :cg], hng, ALU.mult)

                    # sum_h k-tree (k-major: each level adds contiguous
                    # halves, step-1 operands for 2x DVE mode)
                    t1 = work.tile([128, cg // 2], F16, tag="t1", bufs=4)
                    stt(t1[:, :], hng[:, :cg // 2], hng[:, cg // 2:cg],
                        ALU.add)
                    t2 = work.tile([128, cg // 4], F16, tag="t2", bufs=4)
                    stt(t2[:, :], t1[:, :cg // 4], t1[:, cg // 4:cg // 2],
                        ALU.add)
                    stt(sumh[:, gsl], t2[:, :cg // 8], t2[:, cg // 8:cg // 4],
                        ALU.add)
                sumh16 = small.tile([h, cn], F16, tag="sumh16")
                nc.scalar.activation(sumh16[:, :cn], sumh[:, :cn], AF.Copy)

                # sum_gated via 8 accumulated identity matmuls (k-major:
                # each k-slice is a contiguous 112-col block per g)
                psg = ps_sm.tile([h, cn], F32, tag="psg")
                gk = gated[:, :, :].rearrange("p g (kk n) -> p (g kk) n",
                                              kk=k)
                for kk in range(k):
                    nc.tensor.matmul(
                        psg[:, :cn].rearrange("p (g a) -> p g a", g=ng),
                        i16t[:], gk[:, kk::k, :],
                        start=(kk == 0), stop=(kk == k - 1))
                sumg16 = small.tile([h, cn], F16, tag="sumg16")
                nc.scalar.activation(sumg16[:, :cn], psg[:, :cn], AF.Copy)

                # z and pre_h
                psz = ps_sm.tile([h, cn], F32, tag="psz")
                nc.tensor.matmul(psz[:, :cn], w["wz2T"][:],
                                 sumh16[:, :cn],
                                 start=True, stop=False)
                nc.tensor.matmul(psz[:, :cn], i16t[:],
                                 zpre_sb[:, csl],
                                 start=False, stop=True)
                z = small.tile([h, cn], F32, tag="z")
                nc.scalar.activation(z[:, :cn], psz[:, :cn], AF.Sigmoid,
                                     bias=w["bz"][:])

                psh = ps_sm.tile([h, cn], F32, tag="psh")
                nc.tensor.matmul(psh[:, :cn], w["wh2T"][:],
                                 sumg16[:, :cn],
                                 start=True, stop=False)
                nc.tensor.matmul(psh[:, :cn], i16t[:],
                                 hpre_sb[:, csl],
                                 start=False, stop=True)
                ph = small.tile([h, cn], F32, tag="ph")
                nc.scalar.activation(ph[:, :cn], psh[:, :cn], AF.Tanh,
                                     bias=w["bh"][:])

                # h_new = sum_h + z * (pre_h - sum_h) in f32, then f16 image
                t = small.tile([h, cn], F32, tag="tdiff")
                stt(t[:, :cn], ph[:, :cn], sumh[:, :cn], ALU.subtract)
                tz = small.tile([h, cn], F32, tag="tz")
                stt(tz[:, :cn], t[:, :cn], z[:, :cn], ALU.mult)
                hnew = small.tile([h, cn], F32, tag="hnew")
                stt(hnew[:, :cn], tz[:, :cn], sumh[:, :cn], ALU.add)
                if n0 == 0:
                    stt(hnew[:, 0:1], hnew[:, 0:1], w["maskcol"][:], ALU.mult)
                nc.scalar.activation(h16_full[:, csl], hnew[:, :cn], AF.Copy)

                if not last:
                    flush.after_chunk(ci, n0 + cn)
            if last:
                nc.sync.dma_start(out_hT[:, :], h16_full[:, :])
            else:
                flush.finish()


CFG = Cfg()


_PROGRAM = None
LAST_RESULTS = None


def _get_program():
    global _PROGRAM
    if _PROGRAM is None:
        import concourse.bacc as bacc
        import concourse.tile as tile
        nc = bacc.Bacc("TRN2", target_bir_lowering=False, debug=False,
                       num_devices=CFG.n_cores, num_swdge_queues=4)
        ins, out = declare_io(nc, CFG)
        with tile.TileContext(nc) as tc:
            build_gru(tc, out, ins, CFG)
        nc.compile()
        _PROGRAM = nc
    return _PROGRAM


def kernel(fmess, bgraph, W_z, b_z, W_r, U_r, b_Ur, W_h, b_h, **_unused):
    global LAST_RESULTS
    import concourse.bass_utils as bass_utils
    cfg = CFG
    fmess_np = np.asarray(fmess)
    out_dtype = fmess_np.dtype
    in_maps = host_inputs(fmess_np, bgraph, W_z, b_z, W_r, U_r, b_Ur,
                          W_h, b_h, cfg)
    nc = _get_program()
    res = bass_utils.run_bass_kernel_spmd(
        nc, in_maps, core_ids=list(range(cfg.n_cores)))
    LAST_RESULTS = res
    parts = []
    for c in range(cfg.n_cores):
        hT = res.results[c]["hT"]
        parts.append(np.ascontiguousarray(hT[:, :cfg.n_loc].T))
    return np.concatenate(parts, axis=0).astype(out_dtype)


# revision 45
# speedup vs baseline: 1.1349x; 1.1349x over previous
"""DGCN-GRU message passing (nn_DGCNGRU) on 8 Trainium2 NeuronCores.

Strategy (sizes hardcoded for N=50000, K=8, H=128, DEPTH=5, 8 cores):
  - Messages are sharded 6250 rows/core (data-parallel over the message
    dim); the small weights are replicated, all PE matmuls run in fp16.
  - The evolving hidden state h lives in DRAM as a [50000, 128] fp16
    row-major table. Two tables alternate per depth step so the next
    step's table can be AllGathered piecewise (4 pieces per step) while
    the current step's gathers still read the old one.
  - The neighbor gather h[bgraph] uses gpsimd dma_gather with a pair
    trick: index = bgraph>>1 (fits the int16 index limit), each
    descriptor moves 2 rows (512 B = full DMA descriptor efficiency),
    transpose=True lands the two candidate rows as two [128, items]
    SBUF planes; ONE in-place copy_predicated against a uint8 parity
    mask overwrites the lo plane with hi where needed. Gathers are 896
    indices each, 2 per chunk, rotated over 4 SWDGE queues with 6
    chunk-buffers in flight (queue q runs on Q7 core pair q, so 4
    queues generate descriptors in parallel).
  - Compute stays in transposed space [h on partitions, messages on
    the free dim]: r2 = U_r @ h_nei on PE with the depth-invariant r
    precompute added through an accumulated identity matmul;
    sigmoid/tanh on ACT with the torch biases as per-partition bias
    operands; both k-sums (sum_h, sum of r*h_nei) via single DVE
    pool_avg ops with the *8 folded into W_z/W_h host-side.
  - z/h precomputes (W*fmess) stay resident in SBUF across all steps.
  - Step 1 skips the gather entirely (h starts at zero).

kernel(**inputs) takes the full unsharded numpy inputs and returns the
full [50000, 128] float32 output. The Bass program is built and compiled
once per process and reused across calls (it depends only on shapes).
"""


from dataclasses import dataclass

import numpy as np

import concourse.bass as bass
import concourse.mybir as mybir

F16 = mybir.dt.float16
F32 = mybir.dt.float32
U8 = mybir.dt.uint8
I16 = mybir.dt.int16
AF = mybir.ActivationFunctionType
ALU = mybir.AluOpType


@dataclass
class Cfg:
    n_mess: int = 50000
    n_cores: int = 8
    depth: int = 5
    k: int = 8
    h: int = 128
    cn: int = 224          # n's per chunk; 2 gathers of 896 idxs each
    chunks_per_piece: int = 4   # AllGather piece granularity

    @property
    def n_loc(self):
        assert self.n_mess % self.n_cores == 0
        return self.n_mess // self.n_cores

    @property
    def n_pad(self):
        return ((self.n_loc + 127) // 128) * 128

    @property
    def items(self):
        return self.n_pad * self.k

    @property
    def chunks(self):
        """List of (n_offset, cn) chunk descriptors covering n_pad."""
        out = []
        off = 0
        while off < self.n_pad:
            cn = min(self.cn, self.n_pad - off)
            assert (cn * self.k) % 256 == 0
            out.append((off, cn))
            off += cn
        return out

    @property
    def piece_starts(self):
        """Local row start of each AllGather piece (128-aligned). The
        last piece is small so the step-boundary AllGather tail is tiny."""
        step = self.chunks_per_piece * self.cn
        out = [s for s in range(0, self.n_pad - step, step)]
        # split the final group so only a sliver waits on the last chunk
        out.append(self.n_pad - step)
        out.append(((self.n_pad - self.cn) // 128) * 128)
        return out

    @property
    def piece_rows(self):
        """Real (unpadded) local rows per piece."""
        starts = self.piece_starts
        ends = starts[1:] + [self.n_pad]
        return [min(e, self.n_loc) - s for s, e in zip(starts, ends)]

    @property
    def piece_bases(self):
        """Global table base row of each piece (piece-major layout)."""
        out, acc = [], 0
        for pr in self.piece_rows:
            out.append(acc)
            acc += self.n_cores * pr
        return out


def host_inputs(fmess, bgraph, W_z, b_z, W_r, U_r, b_Ur, W_h, b_h, cfg: Cfg):
    """Build the per-core in_map numpy dict."""
    n, h = cfg.n_mess, cfg.h
    nl, npad, k = cfg.n_loc, cfg.n_pad, cfg.k
    fmess = np.asarray(fmess, np.float32)
    bgraph = np.asarray(bgraph)

    shared = {
        "wrT": np.ascontiguousarray(W_r.T).astype(np.float16),
        "wz1T": np.ascontiguousarray(W_z[:, :h].T).astype(np.float16),
        "wh1T": np.ascontiguousarray(W_h[:, :h].T).astype(np.float16),
        "urT": np.ascontiguousarray(U_r.T).astype(np.float16),
        "wz2T": np.ascontiguousarray(W_z[:, h:].T).astype(np.float16),
        "wh2T": np.ascontiguousarray(W_h[:, h:].T).astype(np.float16),
        "ident16": np.eye(h, dtype=np.float16),
        "bz": np.asarray(b_z, np.float32).reshape(h, 1),
        "bur": np.asarray(b_Ur, np.float32).reshape(h, 1),
        "bh": np.asarray(b_h, np.float32).reshape(h, 1),
    }

    # piece-major table row map: the AllGather runs per piece with a
    # contiguous destination, so table rows are ordered
    # [piece0: core0 rows | core1 rows | ...][piece1: ...].
    prows = cfg.piece_rows          # local row count per piece
    pbase = cfg.piece_bases         # global table base per piece
    m = np.arange(n)
    core = m // nl
    r = m % nl
    pidx_of_r = np.searchsorted(np.asarray(cfg.piece_starts), r,
                                side="right") - 1
    off = r - np.asarray(cfg.piece_starts)[pidx_of_r]
    rowmap = (np.asarray(pbase)[pidx_of_r]
              + core * np.asarray(prows)[pidx_of_r] + off)

    in_maps = []
    for c in range(cfg.n_cores):
        sl = slice(c * nl, (c + 1) * nl)
        fT = np.zeros((h, npad), np.float16)
        fT[:, :nl] = fmess[sl].T.astype(np.float16)
        bg = np.zeros((npad, k), np.int64)
        bg[:nl] = rowmap[np.asarray(bgraph[sl])]
        # item stream: per (chunk, gather-half) k-major [k, 112] so the
        # on-chip k-sum trees are contiguous-halves adds (2x DVE mode)
        gn = 112
        flat = bg.reshape(npad // gn, gn, k).transpose(0, 2, 1).reshape(-1)
        pidx = (flat >> 1).astype(np.int16)
        idx = np.tile(pidx.reshape(cfg.items // 16, 16).T, (8, 1))
        mask = np.broadcast_to(
            (flat & 1).astype(np.uint8), (128, cfg.items)).copy()
        maskcol = np.ones((h, 1), np.float32)
        if c == 0:
            maskcol[:, 0] = 0.0
        in_maps.append({
            "fmessT": fT,
            "idx": idx,
            "mask": mask,
            "maskcol": maskcol,
            **shared,
        })
    return in_maps


def declare_io(nc, cfg: Cfg):
    h, npad = cfg.h, cfg.n_pad
    mk = lambda name, shape, dt: nc.dram_tensor(
        name, list(shape), dt, kind="ExternalInput").ap()
    ins = {
        "fmessT": mk("fmessT", (h, npad), F16),
        "idx": mk("idx", (128, cfg.items // 16), I16),
        "mask": mk("mask", (128, cfg.items), U8),
        "maskcol": mk("maskcol", (h, 1), F32),
        "wrT": mk("wrT", (h, h), F16),
        "wz1T": mk("wz1T", (h, h), F16),
        "wh1T": mk("wh1T", (h, h), F16),
        "urT": mk("urT", (h, h), F16),
        "wz2T": mk("wz2T", (h, h), F16),
        "wh2T": mk("wh2T", (h, h), F16),
        "ident16": mk("ident16", (h, h), F16),
        "bz": mk("bz", (h, 1), F32),
        "bur": mk("bur", (h, 1), F32),
        "bh": mk("bh", (h, 1), F32),
    }
    out = nc.dram_tensor("hT", [h, npad], F16, kind="ExternalOutput").ap()
    return ins, out


def build_gru(tc, out_hT, ins, cfg: Cfg):
    nc = tc.nc
    h, k, npad, nl = cfg.h, cfg.k, cfg.n_pad, cfg.n_loc
    N = cfg.n_mess
    rg = [list(range(cfg.n_cores))]
    chunks = cfg.chunks
    cpp = cfg.chunks_per_piece

    pstarts = cfg.piece_starts
    pends = pstarts[1:] + [npad]
    pbases = cfg.piece_bases
    prows = cfg.piece_rows

    # internal DRAM: double-buffered h table (piece-major row order)
    tables = [
        nc.dram_tensor(f"table{i}", [N, h], F16, kind="Internal",
                       addr_space="Shared").ap()
        for i in range(2)
    ]
    shards = [
        nc.dram_tensor(f"shard{i}", [nl, h], F16, kind="Internal").ap()
        for i in range(2)
    ]
    tab_pairs = [t.rearrange("(p two) hh -> p (two hh)", two=2) for t in tables]

    with (
        tc.tile_pool(name="stat", bufs=1) as stat,
        tc.tile_pool(name="work", bufs=1) as work,
        tc.tile_pool(name="small", bufs=2) as small,
        tc.tile_pool(name="ps_big", bufs=1, space="PSUM") as ps_big,
        tc.tile_pool(name="ps_sm", bufs=1, space="PSUM") as ps_sm,
        tc.tile_pool(name="ps_tr", bufs=2, space="PSUM") as ps_tr,
    ):
        # ---- resident statics ----
        fm_sb = stat.tile([h, npad], F16)
        nc.sync.dma_start(fm_sb[:], ins["fmessT"][:])
        idx_sb = stat.tile([128, cfg.items // 16], I16)
        nc.sync.dma_start(idx_sb[:], ins["idx"][:])
        mask_sb = stat.tile([128, cfg.items], U8)
        nc.sync.dma_start(mask_sb[:], ins["mask"][:])
        rpre_sb = stat.tile([h, npad], F16)
        zpre_sb = stat.tile([h, npad], F16)
        hpre_sb = stat.tile([h, npad], F16)
        h16_full = stat.tile([h, npad], F16)
        w = {}
        for name in ("wrT", "wz1T", "wh1T", "wz2T", "wh2T", "urT",
                     "ident16"):
            w[name] = stat.tile([h, h], F16, tag=name, name=name)
            nc.sync.dma_start(w[name][:], ins[name][:])
        for name in ("bz", "bur", "bh", "maskcol"):
            w[name] = stat.tile([h, 1], F32, tag=name, name=name)
            nc.sync.dma_start(w[name][:], ins[name][:])
        urT, i16t = w["urT"], w["ident16"]

        def stt(out, in0, in1, op1, scalar=0.0, op0=ALU.bypass):
            nc.vector.scalar_tensor_tensor(out, in0, scalar, in1, op0, op1)

        def piece_collective(step, pi):
            """AllGather a shard piece into this step's write table."""
            tbl = step % 2
            r0, r1 = pstarts[pi], min(pends[pi], nl)
            gb = pbases[pi]
            nc.gpsimd.collective_compute(
                "AllGather", ALU.bypass, replica_groups=rg,
                ins=[shards[tbl][r0:r1, :]],
                outs=[tables[tbl][gb:gb + cfg.n_cores * prows[pi], :]])

        class StepFlush:
            """Streams h16_full into the shard (PE transpose + DMA per
            128-block as columns complete) and dispatches each piece's
            AllGather two chunks after its rows are all DMA'd (so the
            in-order Pool queue hits an already-satisfied wait)."""

            def __init__(self, step):
                self.step = step
                self.next_b = 0      # next 128-col transpose block
                self.next_p = 0      # next piece to queue
                self.pending = []    # (dispatch_at_chunk, piece)

            def after_chunk(self, ci, done_cols):
                tbl = self.step % 2
                while self.pending and ci >= self.pending[0][0]:
                    piece_collective(self.step, self.pending.pop(0)[1])
                while (self.next_b + 1) * 128 <= done_cols:
                    b = self.next_b * 128
                    pst = ps_tr.tile([128, 1024], F16)
                    nc.tensor.transpose(pst[:, :128],
                                        h16_full[:, b:b + 128], i16t[:])
                    row = small.tile([128, 128], F16, tag="row")
                    nc.scalar.activation(row[:], pst[:, :128], AF.Copy)
                    rows = max(0, min(nl - b, 128))
                    if rows:
                        nc.sync.dma_start(shards[tbl][b:b + rows, :],
                                          row[:rows, :])
                    self.next_b += 1
                while (self.next_p < len(pstarts)
                       and min(pends[self.next_p], nl) <= self.next_b * 128):
                    self.pending.append((ci + 2, self.next_p))
                    self.next_p += 1

            def finish(self):
                for _, pi in self.pending:
                    piece_collective(self.step, pi)
                self.pending = []
                while self.next_p < len(pstarts):
                    piece_collective(self.step, self.next_p)
                    self.next_p += 1

        # ---- phase 0: precomputes + step 1 (h == 0 before the first step) ----
        flush0 = StepFlush(0)
        for ci, (n0, cn) in enumerate(chunks):
            csl = slice(n0, n0 + cn)
            fr = fm_sb[:, csl]

            ps = ps_sm.tile([h, 512], F32, tag="psg")
            nc.tensor.matmul(ps[:, :cn], w["wrT"][:], fr,
                             start=True, stop=True)
            nc.scalar.activation(rpre_sb[:, csl], ps[:, :cn], AF.Copy)

            psz = ps_sm.tile([h, 512], F32, tag="psz")
            nc.tensor.matmul(psz[:, :cn], w["wz1T"][:], fr,
                             start=True, stop=True)
            nc.scalar.activation(zpre_sb[:, csl], psz[:, :cn], AF.Copy)
            z1 = small.tile([h, cn], F16, tag="z")
            nc.scalar.activation(z1[:, :cn], psz[:, :cn], AF.Sigmoid,
                                 bias=w["bz"][:])

            psh = ps_sm.tile([h, 512], F32, tag="psh")
            nc.tensor.matmul(psh[:, :cn], w["wh1T"][:], fr,
                             start=True, stop=True)
            nc.scalar.activation(hpre_sb[:, csl], psh[:, :cn], AF.Copy)
            ph1 = small.tile([h, cn], F16, tag="ph")
            nc.scalar.activation(ph1[:, :cn], psh[:, :cn], AF.Tanh,
                                 bias=w["bh"][:])

            stt(h16_full[:, csl], z1[:, :cn], ph1[:, :cn], ALU.mult)
            if n0 == 0:
                stt(h16_full[:, 0:1], h16_full[:, 0:1], w["maskcol"][:],
                    ALU.mult)
            flush0.after_chunk(ci, n0 + cn)
        flush0.finish()

        # ---- depth steps 2..depth ----
        gq = [0]
        for step in range(1, cfg.depth):
            last = step == cfg.depth - 1
            src = tab_pairs[(step - 1) % 2]
            flush = StepFlush(step)
            for ci, (n0, cn) in enumerate(chunks):
                csl = slice(n0, n0 + cn)
                citems = cn * k
                ioff = n0 * k

                cg = 896
                gn = 112
                ng = cn // gn
                pairs = []
                for g in range(ng):
                    pg = work.tile([128, 2, cg], F16, tag=f"pair{g}", bufs=4)
                    pairs.append(pg)
                    nc.gpsimd.dma_gather(
                        out_ap=pg[:, :, :],
                        in_ap=src,
                        idxs_ap=idx_sb[:, (ioff + g * cg) // 16:
                                       (ioff + (g + 1) * cg) // 16],
                        num_idxs=cg,
                        num_idxs_reg=cg,
                        elem_size=2 * h,
                        transpose=True,
                        queue_num=gq[0] % 4,
                    )
                    gq[0] += 1

                sumh = small.tile([h, cn], F32, tag="sumh")
                gated = work.tile([128, ng, cg], F16, tag="gated", bufs=2)
                for g in range(ng):
                    pg = pairs[g]
                    gsl = slice(g * gn, (g + 1) * gn)
                    # parity select, in place: lo := where(mask, hi, lo)
                    nc.vector.copy_predicated(
                        pg[:, 0, :],
                        mask_sb[:, ioff + g * cg:ioff + (g + 1) * cg],
                        pg[:, 1, :])
                    hng = pg[:, 0, :]   # [128, cg], k-major: [k=8, n=112]

                    # r2 = rpre (ACT-broadcast into PSUM) + U_r @ hn, sigmoid
                    r16 = work.tile([128, cg], F16, tag=f"r16{g}", bufs=2)
                    rb = rpre_sb[:, n0 + g * gn:n0 + (g + 1) * gn]
                    rb = rb.rearrange("p (one a) -> p one a", one=1)
                    rb = rb.broadcast_to((128, 4, gn))
                    for s0 in (0, cg // 2):
                        psr = ps_big.tile([128, 512], F32, tag="psr",
                                          bufs=2)
                        nc.tensor.matmul(
                            psr[:, :448], urT[:], hng[:, s0:s0 + cg // 2],
                            start=True, stop=False)
                        nc.tensor.matmul(psr[:, :448], i16t[:], rb,
                                         start=False, stop=True)
                        nc.scalar.activation(r16[:, s0:s0 + cg // 2],
                                             psr[:, :448],
                                             AF.Sigmoid, bias=w["bur"][:])

                    stt(gated[:, g, :], r16[:, :cg], hng, ALU.mult)

                    # sum_h k-tree (k-major: each level adds contiguous
                    # halves, step-1 operands for 2x DVE mode)
                    t1 = work.tile([128, cg // 2], F16, tag="t1", bufs=4)
                    stt(t1[:, :], hng[:, :cg // 2], hng[:, cg // 2:cg],
                        ALU.add)
                    t2 = work.tile([128, cg // 4], F16, tag="t2", bufs=4)
                    stt(t2[:, :], t1[:, :cg // 4], t1[:, cg // 4:cg // 2],
                        ALU.add)
                    stt(sumh[:, gsl], t2[:, :cg // 8], t2[:, cg // 8:cg // 4],
                        ALU.add)
                sumh16 = small.tile([h, cn], F16, tag="sumh16")
                nc.scalar.activation(sumh16[:, :cn], sumh[:, :cn], AF.Copy)

                # sum_gated via 8 accumulated identity matmuls (k-major:
                # each k-slice is a contiguous 112-col block per g)
                psg = ps_sm.tile([h, 512], F32, tag="psg")
                gk = gated[:, :, :].rearrange("p g (kk n) -> p (g kk) n",
                                              kk=k)
                for kk in range(k):
                    nc.tensor.matmul(
                        psg[:, :cn].rearrange("p (g a) -> p g a", g=ng),
                        i16t[:], gk[:, kk::k, :],
                        start=(kk == 0), stop=(kk == k - 1))
                sumg16 = small.tile([h, cn], F16, tag="sumg16")
                nc.scalar.activation(sumg16[:, :cn], psg[:, :cn], AF.Copy)

                # z and pre_h
                psz = ps_sm.tile([h, 512], F32, tag="psz")
                nc.tensor.matmul(psz[:, :cn], w["wz2T"][:],
                                 sumh16[:, :cn],
                                 start=True, stop=False)
                nc.tensor.matmul(psz[:, :cn], i16t[:],
                                 zpre_sb[:, csl],
                                 start=False, stop=True)
                z = small.tile([h, cn], F32, tag="z")
                nc.scalar.activation(z[:, :cn], psz[:, :cn], AF.Sigmoid,
                                     bias=w["bz"][:])

                psh = ps_sm.tile([h, 512], F32, tag="psh")
                nc.tensor.matmul(psh[:, :cn], w["wh2T"][:],
                                 sumg16[:, :cn],
                                 start=True, stop=False)
                nc.tensor.matmul(psh[:, :cn], i16t[:],
                                 hpre_sb[:, csl],
                                 start=False, stop=True)
                ph = small.tile([h, cn], F32, tag="ph")
                nc.scalar.activation(ph[:, :cn], psh[:, :cn], AF.Tanh,
                                     bias=w["bh"][:])

                # h_new = sum_h + z * (pre_h - sum_h) in f32, then f16 image
                t = small.tile([h, cn], F32, tag="tdiff")
                stt(t[:, :cn], ph[:, :cn], sumh[:, :cn], ALU.subtract)
                tz = small.tile([h, cn], F32, tag="tz")
                stt(tz[:, :cn], t[:, :cn], z[:, :cn], ALU.mult)
                hnew = small.tile([h, cn], F32, tag="hnew")
                stt(hnew[:, :cn], tz[:, :cn], sumh[:, :cn], ALU.add)
                if n0 == 0:
                    stt(hnew[:, 0:1], hnew[:, 0:1], w["maskcol"][:], ALU.mult)
                nc.scalar.activation(h16_full[:, csl], hnew[:, :cn], AF.Copy)

                if not last:
                    flush.after_chunk(ci, n0 + cn)
            if last:
                nc.sync.dma_start(out_hT[:, :], h16_full[:, :])
            else:
                flush.finish()


CFG = Cfg()


_PROGRAM = None
LAST_RESULTS = None


def _get_program():
    global _PROGRAM
    if _PROGRAM is None:
        import concourse.bacc as bacc
        import concourse.tile as tile
        nc = bacc.Bacc("TRN2", target_bir_lowering=False, debug=False,
                       num_devices=CFG.n_cores, num_swdge_queues=4)
        ins, out = declare_io(nc, CFG)
        with tile.TileContext(nc) as tc:
            build_gru(tc, out, ins, CFG)
        nc.compile()
        _PROGRAM = nc
    return _PROGRAM


def kernel(fmess, bgraph, W_z, b_z, W_r, U_r, b_Ur, W_h, b_h, **_unused):
    global LAST_RESULTS
    import concourse.bass_utils as bass_utils
    cfg = CFG
    fmess_np = np.asarray(fmess)
    out_dtype = fmess_np.dtype
    in_maps = host_inputs(fmess_np, bgraph, W_z, b_z, W_r, U_r, b_Ur,
                          W_h, b_h, cfg)
    nc = _get_program()
    res = bass_utils.run_bass_kernel_spmd(
        nc, in_maps, core_ids=list(range(cfg.n_cores)))
    LAST_RESULTS = res
    parts = []
    for c in range(cfg.n_cores):
        hT = res.results[c]["hT"]
        parts.append(np.ascontiguousarray(hT[:, :cfg.n_loc].T))
    return np.concatenate(parts, axis=0).astype(out_dtype)
